# revision 7
# baseline (speedup 1.0000x reference)
"""SCL (supervised contrastive) loss on 8 Trainium2 NeuronCores.

Strategy:
  - Host: sort the 2N rows by label so each row's same-label block (positives
    + diagonal) is one contiguous column range; pre-scale features by
    sqrt(1/T) so the device matmul directly produces logits. Rotate each
    core's column space by -core*rpc so every core's diagonal block sits at
    the same (compile-time) chunk positions -> one SPMD program.
  - Device (SPMD over 8 cores, 1024 rows each): per [128 x 2048] chunk of
    (row-block x all-columns):
      PE  : 4x f32r matmuls (K=128) -> PSUM; plus low-rank bf16 indicator
            matmuls (A_row^T @ A_col, values +-448 -> -200704) accumulated
            into PSUM to push each row's same-label columns to ~-2e5.
      DVE : tensor_reduce(max, negate=True) over the chunk -> -max per row
      ACT : Exp activation, bias=-max, accum_out -> sum(exp) per row
    Per-(row, chunk) outputs: -max and sum(exp) -- 2 x [128, 32] per core.
  - Host: merge per-chunk (max, sumexp) into per-row logsumexp over
    negatives in fp64; add the sparse positive-pair softplus terms from the
    small within-group Gram blocks; average into the scalar loss.
"""

import numpy as np


def _pin_b16_neuronxcc():
    """Make `import neuronxcc` resolve to the b16-bazel build that matches
    this concourse (the default env's neuronxcc rejects bass BIR with
    'Reg has not been allocated yet')."""
    import glob
    import os
    import sys

    if "neuronxcc" in sys.modules:
        return
    cands = [p for p in os.environ.get("NIX_PYTHONPATH", "").split(os.pathsep)
             if p and os.path.isdir(os.path.join(p, "neuronxcc"))]
    cands += [os.path.dirname(p) for p in
              glob.glob("/nix/store/*b16-bazel*/lib/python*/site-packages/neuronxcc")]
    b16 = next((p for p in cands if "b16" in p), None)
    if b16 is None:
        return
    sys.path.insert(0, b16)
    try:
        import neuronxcc  # noqa: F401  # binds neuronxcc.__path__ to b16
    finally:
        sys.path.remove(b16)


_pin_b16_neuronxcc()

TEMPERATURE = 0.1
NCORES = 8
PART = 128          # hardware partitions / feature dim
MM_N = 512          # moving-operand cols per f32 matmul (one PSUM bank)
CHUNK = 2048        # columns per DVE/ACT chunk (4 PSUM banks)
WIN = 1024          # per-tile mask window (2 PSUM banks), 512-aligned
IND = 448.0         # bf16 indicator magnitude; 448^2 = 200704 >> max |logit|
MAXG = 96           # group-size bound the device layout assumes

# set by test harness for profiling; kernel() publishes results here
TRACE = False
TRACE_ALL_CORES = True
LAST_RESULTS = None


def _win_start(t, rpc_chunk):
    """512-aligned start of the mask window for row-tile t (rotated cols)."""
    w = 512 * ((128 * t - MAXG) // 512)
    return int(min(max(w, 0), rpc_chunk - WIN))


def _build_program(M, rpc, chunk):
    """Build the per-core Bass program (shared by all cores)."""
    from contextlib import ExitStack

    import concourse.tile as tile
    from concourse import bacc, mybir

    tpc = rpc // PART           # row-tiles per core
    kch = M // chunk            # chunks per row
    nstat = tpc * kch
    f32 = mybir.dt.float32
    f32r = mybir.dt.float32r
    bf16 = mybir.dt.bfloat16

    nc = bacc.Bacc("TRN2", target_bir_lowering=False, debug=False)

    fT_d = nc.dram_tensor("fT", [PART, M], f32r, kind="ExternalInput")
    arow_d = nc.dram_tensor("arow", [PART, rpc], bf16, kind="ExternalInput")
    acols_d = nc.dram_tensor("acols", [PART, tpc * WIN], bf16,
                             kind="ExternalInput")
    aroww_d = nc.dram_tensor("aroww", [PART, PART], bf16, kind="ExternalInput")
    acolsw_d = nc.dram_tensor("acolsw", [PART, MM_N], bf16,
                              kind="ExternalInput")
    negm_d = nc.dram_tensor("negm", [PART, nstat], f32, kind="ExternalOutput")
    sums_d = nc.dram_tensor("sums", [PART, nstat], f32, kind="ExternalOutput")

    with tile.TileContext(nc) as tc, ExitStack() as ctx:
        consts = ctx.enter_context(tc.tile_pool(name="consts", bufs=1))
        psum = ctx.enter_context(tc.tile_pool(name="psum", bufs=2, space="PSUM"))
        scrp = ctx.enter_context(tc.tile_pool(name="scr", bufs=2))
        statp = ctx.enter_context(tc.tile_pool(name="stat", bufs=8))

        fT = consts.tile([PART, M], f32r)
        nc.sync.dma_start(fT[:], fT_d[:])
        arow = consts.tile([PART, rpc], bf16)
        nc.sync.dma_start(arow[:], arow_d[:])
        acols = consts.tile([PART, tpc * WIN], bf16)
        nc.sync.dma_start(acols[:], acols_d[:])
        aroww = consts.tile([PART, PART], bf16)
        nc.sync.dma_start(aroww[:], aroww_d[:])
        acolsw = consts.tile([PART, MM_N], bf16)
        nc.sync.dma_start(acolsw[:], acolsw_d[:])

        for t in range(tpc):
            w_t = _win_start(t, chunk)
            for k in range(kch):
                idx = t * kch + k
                pt = psum.tile([PART, chunk], f32)
                for j in range(chunk // MM_N):
                    c0 = k * chunk + j * MM_N
                    nc.tensor.matmul(
                        pt[:, j * MM_N:(j + 1) * MM_N],
                        fT[:, t * PART:(t + 1) * PART],
                        fT[:, c0:c0 + MM_N],
                        start=True,
                        stop=True,
                    )
                if k == 0:
                    # mask this tile's same-label block: psum += arow^T@acols
                    for j2 in range(WIN // MM_N):
                        nc.tensor.matmul(
                            pt[:, w_t + j2 * MM_N:w_t + (j2 + 1) * MM_N],
                            arow[:, t * PART:(t + 1) * PART],
                            acols[:, t * WIN + j2 * MM_N:
                                  t * WIN + (j2 + 1) * MM_N],
                            start=False,
                            stop=True,
                            skip_group_check=True,
                        )
                if k == kch - 1 and t == 0:
                    # wrap tail of the boundary-straddling group
                    nc.tensor.matmul(
                        pt[:, chunk - MM_N:chunk],
                        aroww[:],
                        acolsw[:],
                        start=False,
                        stop=True,
                        skip_group_check=True,
                    )
                nm = statp.tile([PART, 1], f32, tag="nm")
                sm = statp.tile([PART, 1], f32, tag="sm")
                nc.vector.tensor_reduce(
                    out=nm[:], in_=pt[:], axis=mybir.AxisListType.X,
                    op=mybir.AluOpType.max, negate=True,
                )
                sc = scrp.tile([PART, chunk], f32)
                nc.scalar.activation(
                    out=sc[:], in_=pt[:],
                    func=mybir.ActivationFunctionType.Exp,
                    bias=nm[:], scale=1.0, accum_out=sm[:],
                )
                nc.sync.dma_start(negm_d[:, idx:idx + 1], nm[:])
                nc.sync.dma_start(sums_d[:, idx:idx + 1], sm[:])

    nc.finalize()
    return nc


def _prep_inputs(features, label, ncores, chunk):
    """Sort by label, rotate per core, build input maps + host group info."""
    import ml_dtypes

    f = np.asarray(features, np.float32).reshape(-1, PART)
    M = f.shape[0]
    lab = np.repeat(np.asarray(label).ravel(), 2)
    assert lab.shape[0] == M
    rpc = M // ncores
    tpc = rpc // PART

    perm = np.argsort(lab, kind="stable")
    lab_s = lab[perm]
    f_s = f[perm] * np.float32(np.sqrt(1.0 / TEMPERATURE))
    fT = np.ascontiguousarray(f_s.T)  # [PART, M]

    # group range [g_lo[r], g_hi[r]) for each sorted row
    change = np.r_[True, lab_s[1:] != lab_s[:-1]]
    gid = np.cumsum(change) - 1
    starts = np.flatnonzero(change)
    ends = np.r_[starts[1:], M]
    g_lo = starts[gid].astype(np.int64)
    g_hi = ends[gid].astype(np.int64)
    if int((ends - starts).max()) > MAXG:
        raise _FallbackToHost  # device layout assumes bounded group size

    in_maps = []
    for c in range(ncores):
        r0 = rpc * c
        fT_c = np.roll(fT, -r0, axis=1)
        rows = r0 + np.arange(rpc)
        lo = g_lo[rows] - r0           # rotated-frame group start (may be <0)
        hi = g_hi[rows] - r0
        arow = np.zeros((PART, rpc), np.float32)
        acols = np.zeros((PART, tpc * WIN), np.float32)
        aroww = np.zeros((PART, PART), np.float32)
        acolsw = np.zeros((PART, MM_N), np.float32)
        for t in range(tpc):
            w_t = _win_start(t, chunk)
            slot_of = {}
            for p in range(PART):
                r = 128 * t + p
                key = (lo[r], hi[r])
                s = slot_of.get(key)
                if s is None:
                    s = len(slot_of)
                    slot_of[key] = s
                    # columns of this group (non-wrap part), window-relative
                    assert s < PART, "too many groups in one row-tile"
                    c_lo, c_hi = max(int(lo[r]), 0), int(hi[r])
                    assert w_t <= c_lo and c_hi <= w_t + WIN, \
                        (t, c_lo, c_hi, w_t)
                    acols[s, t * WIN + (c_lo - w_t):
                          t * WIN + (c_hi - w_t)] = -IND
                    if lo[r] < 0:      # wrap tail -> last 512 cols of row
                        assert t == 0 and -lo[r] <= MM_N
                        acolsw[0, MM_N + int(lo[r]):] = -IND
                arow[s, r] = IND
                if lo[r] < 0:
                    aroww[0, r] = IND
        in_maps.append({
            "fT": fT_c,
            "arow": arow.astype(ml_dtypes.bfloat16),
            "acols": acols.astype(ml_dtypes.bfloat16),
            "aroww": aroww.astype(ml_dtypes.bfloat16),
            "acolsw": acolsw.astype(ml_dtypes.bfloat16),
        })

    host = {
        "perm": perm, "f_s": f_s, "starts": starts, "ends": ends,
        "M": M, "rpc": rpc, "tpc": tpc, "kch": M // chunk,
    }
    return in_maps, host


class _FallbackToHost(Exception):
    pass


def _finish_host(negm_list, sums_list, host, ncores):
    """fp64 merge of per-chunk stats + sparse positive-pair terms."""
    M, tpc, kch = host["M"], host["tpc"], host["kch"]
    negm = np.stack(negm_list)  # [ncores, PART, tpc*kch]
    sums = np.stack(sums_list)
    mc = -negm.reshape(ncores, PART, tpc, kch).transpose(0, 2, 1, 3) \
        .reshape(M, kch).astype(np.float64)
    sc = sums.reshape(ncores, PART, tpc, kch).transpose(0, 2, 1, 3) \
        .reshape(M, kch).astype(np.float64)

    M_r = mc.max(axis=1)
    with np.errstate(invalid="ignore", over="ignore"):
        S_r = np.where(mc > -1e38, sc * np.exp(mc - M_r[:, None]), 0.0).sum(1)
    with np.errstate(divide="ignore"):
        neg_lse = np.where(S_r > 0.0, M_r + np.log(np.maximum(S_r, 1e-300)),
                           -np.inf)

    # positives: within-group Gram blocks (f_s already scaled by sqrt(1/T))
    f64 = host["f_s"].astype(np.float64)
    row_loss = np.zeros(M)
    for lo, hi in zip(host["starts"], host["ends"]):
        g = hi - lo
        if g <= 1:
            continue  # no positives (reference can't hit this: paired views)
        B = f64[lo:hi] @ f64[lo:hi].T  # [g, g] logits
        t = neg_lse[lo:hi, None] - B
        sp = np.logaddexp(0.0, t)  # softplus, handles -inf -> 0
        np.fill_diagonal(sp, 0.0)
        row_loss[lo:hi] = sp.sum(axis=1) / (g - 1)

    return np.float32(row_loss.sum() / M)


def _host_reference(features, label):
    """Pure-host fp64 fallback (pathological label distributions only)."""
    f = np.asarray(features, np.float64).reshape(-1, PART)
    M = f.shape[0]
    logit = (f @ f.T) / TEMPERATURE
    lab2 = np.repeat(np.asarray(label).ravel(), 2)
    same = lab2[:, None] == lab2[None, :]
    np.fill_diagonal(same, True)
    neg = np.where(~same, logit, -np.inf)
    m = neg.max(axis=1)
    m_safe = np.where(np.isfinite(m), m, 0.0)
    with np.errstate(over="ignore", invalid="ignore"):
        s = np.exp(np.where(~same, logit - m_safe[:, None], -np.inf)).sum(1)
    with np.errstate(divide="ignore"):
        neg_lse = np.where(s > 0, m_safe + np.log(np.maximum(s, 1e-300)),
                           -np.inf)
    is_pos = same.copy()
    np.fill_diagonal(is_pos, False)
    t = neg_lse[:, None] - logit
    sp = np.logaddexp(0.0, t)
    cnt = is_pos.sum(1)
    row = np.where(is_pos, sp, 0.0).sum(1) / np.maximum(cnt, 1)
    return np.float32(row.sum() / M)


def kernel(features, label):
    global LAST_RESULTS
    from concourse.bass_utils import run_bass_kernel_spmd

    f = np.asarray(features, np.float32)
    M = f.shape[0] * 2
    assert f.shape[-1] == PART and M % (NCORES * PART) == 0 and M % CHUNK == 0

    try:
        in_maps, host = _prep_inputs(features, label, NCORES, CHUNK)
    except _FallbackToHost:
        return _host_reference(features, label)

    nc = _build_program(M, host["rpc"], CHUNK)

    kwargs = {}
    if TRACE:
        kwargs = dict(trace=True)
        if TRACE_ALL_CORES:
            kwargs["trace_cores"] = list(range(NCORES))
    res = run_bass_kernel_spmd(nc, in_maps, list(range(NCORES)), **kwargs)
    LAST_RESULTS = res

    negm_list = [res.results[c]["negm"] for c in range(NCORES)]
    sums_list = [res.results[c]["sums"] for c in range(NCORES)]
    return _finish_host(negm_list, sums_list, host, NCORES)
